# revision 6
# baseline (speedup 1.0000x reference)
"""ConvEncoder kernel for 8 TRN2 NeuronCores (raw Bacc, manual semaphores).

Computes: emb = emb_table[x]; windows = im2col(pad(emb), WIN=5);
y = gelu(windows @ W.T + b), for x (16, 2048) int32 ids.
Sharding: data-parallel over batch - 2 batches per core x 8 cores; table
and weights replicated per core (host pre-casts both to bf16; gelu output
is stored bf16 and upcast to f32 on host).

Gather: gpsimd dma_gather(transpose=True) writes table rows directly as
embT columns (EMB on partitions), eliminating PE transposes + PSUM copies.
Its int16 indices only cover 15 bits, so the host lays out a 65536-row
wrapped table T2 = [tbl[32768:] | pad | tbl[:32768]] with the in_ap based
at row 32768: the Q7 descgen's sign-extended index then lands on
T2[32768 + signed16(id)] == tbl[id] for the full vocab. The Q7 kernel
drops a trailing run of negative (>=32768) indices, so every batch is
gathered as chunk A = tokens [0,1152) and chunk B = tokens [1024,2048)
plus 128 pad indices of id 0: B re-covers A's droppable tail, and B's own
tail is the never-negative pad (id 0 is the all-zero PAD row, which also
writes the right-halo zeros).

Engine programs per core:
  sync:   idx/weights/bias loads, then 5 chunked output stores
  gpsimd: 4 transpose-gathers (A0 B0 A1 B1) into embT
  tensor: PE-clock warmup matmuls, then 8 spans x 5-tap matmuls
  vector: left-halo + scratch memsets
  scalar: gelu-table preload, then exact GELU(+bias) PSUM->bf16 ring
"""

import numpy as np

import concourse.bass as bass
import concourse.mybir as mybir
from concourse import bacc
from concourse.bass_utils import run_bass_kernel_spmd

B, S, EMB, WIN, OUT, VOCAB = 16, 2048, 128, 5, 128, 50257
NCORES = 8
BPC = B // NCORES              # 2 batches/core
T = BPC * S                    # 4096 tokens/core
SPAN = 512
NSPAN = T // SPAN              # 8
SPB = S // SPAN                # 4 spans per batch
HALO = WIN // 2
HALFV = 32768
PAD = 128                      # pad idxs appended to each batch's chunk B
ALEN = 1152                    # chunk A tokens [0, ALEN)
BOFF = 1024                    # chunk B tokens [BOFF, S) + PAD zeros
BLEN = S - BOFF + PAD          # 1152
PADBASE = 128                  # cols before batch 0 (last 2 are its left halo)
BSTRIDE = S + PAD              # embT columns per batch (2176, 128-aligned)
NIDX = ALEN + BLEN             # idxs per batch (2304)

BF16 = mybir.dt.bfloat16

NPS = 3                        # matmul psum banks

# per-span gather dependencies: indices into the 4 per-gather semaphores
# (A0, B0, A1, B1). q0 needs A; q1,q2 straddle A and B; q3 needs B only.
SPAN_SG = [(0,), (0, 1), (0, 1), (1,), (2,), (2, 3), (2, 3), (3,)]

# stores: (out col start, width, s_act needed, ao ring offset)
STORES = [
    (0, 1024, 2, 0),
    (1024, 1024, 4, 1024),
    (2048, 1024, 6, 0),
    (3072, 512, 7, 1024),
    (3584, 512, 8, 1536),
]
# gelu span j must wait for the store that last read its ao segment
AO_WAIT = {4: 16, 5: 16, 6: 32, 7: 32}

# PE clock warmup: dummy matmuls on a zeroed scratch tile keep the PE
# continuously busy from ~0.4us so it reaches the 2.4GHz p-state before
# the first gathered span lands.
N_WARM_BIG = 36     # 256-col dummies
N_WARM_SMALL = 48   # 64-col dummies

_cache = {}


def _build():
    nc = bacc.Bacc("TRN2", target_bir_lowering=False, debug=False)
    xi = nc.declare_dram_parameter(
        "xi", [128, BPC * NIDX // 16], mybir.dt.int16, isOutput=False
    )
    t2 = nc.declare_dram_parameter("t2", [2 * HALFV, EMB], BF16, isOutput=False)
    cst = nc.declare_dram_parameter("cst", [128, WIN * OUT], BF16, isOutput=False)
    bv = nc.declare_dram_parameter("bias", [128, 1], mybir.dt.float32, isOutput=False)
    out = nc.declare_dram_parameter("out", [128, T], BF16, isOutput=True)

    idx_sb = nc.alloc_sbuf_tensor("idx_sb", [128, BPC * NIDX // 16], mybir.dt.int16)
    embT = nc.alloc_sbuf_tensor("embT", [128, 1, PADBASE + BPC * BSTRIDE], BF16)
    cst_sb = nc.alloc_sbuf_tensor("cst_sb", [128, WIN * OUT], BF16)
    b_sb = nc.alloc_sbuf_tensor("b_sb", [128, 1], mybir.dt.float32)
    ao = nc.alloc_sbuf_tensor("ao", [128, 4 * SPAN], BF16)
    scr = nc.alloc_sbuf_tensor("scr", [128, 256], BF16)
    scrb = nc.alloc_sbuf_tensor("scrb", [128, 1], mybir.dt.float32)
    pss = [nc.alloc_psum_tensor(f"ps{i}", [128, SPAN], mybir.dt.float32) for i in range(NPS)]
    psd = nc.alloc_psum_tensor("psd", [128, 256], mybir.dt.float32)

    with (
        nc.semaphore("s_idx") as s_idx,
        nc.semaphore("s_ld") as s_ld,
        nc.semaphore("s_b") as s_b,
        nc.semaphore("s_scr") as s_scr,
        nc.semaphore("s_g0") as s_g0,
        nc.semaphore("s_g1") as s_g1,
        nc.semaphore("s_g2") as s_g2,
        nc.semaphore("s_g3") as s_g3,
        nc.semaphore("s_mm") as s_mm,
        nc.semaphore("s_act") as s_act,
        nc.semaphore("s_out") as s_out,
        nc.Block(no_gpsimd_drain=True) as block,
    ):

        @block.sync
        def _(sync):
            sync.dma_start(out=idx_sb[:], in_=xi[:]).then_inc(s_idx, 16)
            sync.dma_start(out=cst_sb[:], in_=cst[:]).then_inc(s_ld, 16)
            sync.dma_start(out=b_sb[:], in_=bv[:]).then_inc(s_b, 16)
            for c0, w, na, aoff in STORES:
                sync.wait_ge(s_act, na)
                sync.dma_start(
                    out=out[:, c0 : c0 + w], in_=ao[:, aoff : aoff + w]
                ).then_inc(s_out, 16)
            sync.wait_ge(s_out, 16 * len(STORES))

        @block.vector
        def _(vector):
            # left halos; batch>0's also gets zeroed by the previous batch's
            # pad gather (id 0 = zero row), same value either way
            for bb in range(BPC):
                base = PADBASE + bb * BSTRIDE
                nc.vector.memset(embT[:, :, base - HALO : base], 0.0)
            nc.vector.memset(scr[:], 0.0)
            nc.vector.memset(scrb[:], 0.0).then_inc(s_scr, 1)

        @block.gpsimd
        def _(gpsimd):
            gsems = [s_g0, s_g1, s_g2, s_g3]
            gpsimd.wait_ge(s_idx, 16)
            for bb in range(BPC):
                base = PADBASE + bb * BSTRIDE
                io = bb * NIDX // 16
                nc.gpsimd.dma_gather(
                    out_ap=embT[:, :, base : base + ALEN],
                    in_ap=t2[HALFV:, :],
                    idxs_ap=idx_sb[:, io : io + ALEN // 16],
                    num_idxs=ALEN,
                    num_idxs_reg=ALEN,
                    elem_size=EMB,
                    transpose=True,
                    single_packet=False,
                ).then_inc(gsems[2 * bb], 16)
                nc.gpsimd.dma_gather(
                    out_ap=embT[:, :, base + BOFF : base + BOFF + BLEN],
                    in_ap=t2[HALFV:, :],
                    idxs_ap=idx_sb[:, io + ALEN // 16 : io + NIDX // 16],
                    num_idxs=BLEN,
                    num_idxs_reg=BLEN,
                    elem_size=EMB,
                    transpose=True,
                    single_packet=False,
                ).then_inc(gsems[2 * bb + 1], 16)

        @block.tensor
        def _(tensor):
            tensor.wait_ge(s_scr, 1)
            for _ in range(N_WARM_BIG):
                nc.tensor.matmul(
                    out=psd[:, 0:256], lhsT=scr[:, 0:128], rhs=scr[:, 0:256],
                    start=True, stop=True,
                )
            for _ in range(N_WARM_SMALL):
                nc.tensor.matmul(
                    out=psd[:, 0:64], lhsT=scr[:, 0:128], rhs=scr[:, 0:64],
                    start=True, stop=True,
                )
            tensor.wait_ge(s_ld, 16)
            gsems = [s_g0, s_g1, s_g2, s_g3]
            seen = set()
            for j in range(NSPAN):
                for gi in SPAN_SG[j]:
                    if gi not in seen:
                        tensor.wait_ge(gsems[gi], 16)
                        seen.add(gi)
                if j >= NPS:
                    tensor.wait_ge(s_act, j - NPS + 1)  # ps bank free
                base = PADBASE + (j // SPB) * BSTRIDE + (j % SPB) * SPAN - HALO
                ps = pss[j % NPS]
                for k in range(WIN):
                    mm = nc.tensor.matmul(
                        out=ps[:],
                        lhsT=cst_sb[:, k * OUT : (k + 1) * OUT],
                        rhs=embT[:, 0, base + k : base + k + SPAN],
                        start=(k == 0),
                        stop=(k == WIN - 1),
                    )
                mm.then_inc(s_mm, 1)

        @block.scalar
        def _(scalar):
            scalar.wait_ge(s_scr, 1)
            # gelu table preload during the gather head
            nc.scalar.activation(
                out=scrb[:], in_=scrb[:],
                func=mybir.ActivationFunctionType.Gelu, bias=scrb[:, 0:1],
            )
            scalar.wait_ge(s_b, 16)
            for j in range(NSPAN):
                scalar.wait_ge(s_mm, j + 1)
                if j in AO_WAIT:
                    scalar.wait_ge(s_out, AO_WAIT[j])  # ao segment free
                seg = (j % 4) * SPAN
                nc.scalar.activation(
                    out=ao[:, seg : seg + SPAN],
                    in_=pss[j % NPS][:],
                    func=mybir.ActivationFunctionType.Gelu,
                    bias=b_sb[:, 0:1],
                ).then_inc(s_act, 1)

    nc.compile()
    return nc


def _wrap_idx(chunk):
    # dma_gather index layout: idx i lives at (partition i%16, col i//16),
    # replicated across the 8 groups of 16 partitions.
    w = len(chunk) // 16
    return chunk.astype(np.uint16).view(np.int16).reshape(w, 16).T


def _prep_inputs(x, emb_table, W, b):
    import ml_dtypes

    x = np.asarray(x).astype(np.int64)
    tblf = np.asarray(emb_table, dtype=np.float32).astype(ml_dtypes.bfloat16)
    t2 = np.zeros((2 * HALFV, EMB), dtype=ml_dtypes.bfloat16)
    t2[: VOCAB - HALFV] = tblf[HALFV:]
    t2[HALFV:] = tblf[:HALFV]
    W = np.asarray(W, dtype=np.float32)
    wt = W.reshape(OUT, WIN, EMB).transpose(2, 1, 0).reshape(EMB, WIN * OUT)
    cst = np.ascontiguousarray(wt.astype(ml_dtypes.bfloat16))
    bias = np.ascontiguousarray(np.asarray(b, dtype=np.float32).reshape(128, 1))
    in_maps = []
    for core in range(NCORES):
        xc = x[core * BPC : (core + 1) * BPC]
        blocks = []
        for bb in range(BPC):
            tb = xc[bb]
            ca = tb[:ALEN]
            cb = np.concatenate([tb[BOFF:], np.zeros(PAD, dtype=np.int64)])
            blocks.extend([_wrap_idx(ca), _wrap_idx(cb)])
        idx16 = np.concatenate(blocks, axis=1)           # [16, BPC*NIDX/16]
        idx128 = np.ascontiguousarray(np.tile(idx16, (8, 1)))
        in_maps.append({"xi": idx128, "t2": t2, "cst": cst, "bias": bias})
    return in_maps


def kernel(x, emb_table, W, b, _trace=False):
    if "nc" not in _cache:
        _cache["nc"] = _build()
    nc = _cache["nc"]
    in_maps = _prep_inputs(x, emb_table, W, b)
    res = run_bass_kernel_spmd(nc, in_maps, core_ids=list(range(NCORES)), trace=_trace)
    _cache["last_result"] = res
    outs = []
    for core in range(NCORES):
        oc = np.asarray(res.results[core]["out"]).astype(np.float32)
        outs.append(oc.T.reshape(BPC, S, OUT))
    return np.concatenate(outs, axis=0)


# revision 7
# speedup vs baseline: 1.1417x; 1.1417x over previous
"""ConvEncoder kernel for 8 TRN2 NeuronCores (raw Bacc, manual semaphores).

Computes: emb = emb_table[x]; windows = im2col(pad(emb), WIN=5);
y = gelu(windows @ W.T + b), for x (16, 2048) int32 ids.
Sharding: data-parallel over batch - 2 batches per core x 8 cores; the
embedding table (f32) and bf16 weights are replicated per core.

The gather is Q7-descgen-bound (~10ns/row on every SWDGE path), so the
pipeline is built to keep the 32 per-tile indirect gathers streaming
back-to-back from the moment the ids land, with everything else hidden
under them:
  sync:   idx load first, then wt/bias/ident; per-piece output stores
  gpsimd: 32 indirect row-gathers (f32 rows -> bf16 in flight), offsets
          sliced directly from idx_sb (no per-column copies to wait on)
  tensor: 32 bf16 transposes + 5-tap matmuls per span (PSUM f32)
  vector: embT halo memsets, PSUM->embT copies
  scalar: exact GELU(+bias) PSUM -> bf16 ring (host upcasts to f32)
"""

import numpy as np

import concourse.bass as bass
import concourse.mybir as mybir
from concourse import bacc
from concourse.bass import IndirectOffsetOnAxis
from concourse.bass_utils import run_bass_kernel_spmd

B, S, EMB, WIN, OUT, VOCAB = 16, 2048, 128, 5, 128, 50257
NCORES = 8
BPC = B // NCORES
T = BPC * S                    # 4096 tokens/core
NTILE = T // 128               # 32
TPB = S // 128                 # 16
SPAN = 512
NSPAN = T // SPAN              # 8
SPB = S // SPAN                # 4
HALO = WIN // 2
EC = S + 2 * HALO              # 2052

MM_DT = mybir.dt.bfloat16
NPT = 5                        # transpose psum banks
NPS = 3                        # matmul psum banks / ao buffers

# output pieces: spans 0..6 full, span 7 split in halves
# (j, col offset within span, width, last embT tile needed)
PIECES = []
for j in range(NSPAN - 1):
    need = min(4 * j + 4, ((j // SPB) + 1) * TPB - 1)
    PIECES.append((j, 0, SPAN, need))
PIECES.append((NSPAN - 1, 0, SPAN // 2, 30))
PIECES.append((NSPAN - 1, SPAN // 2, SPAN // 2, 31))
PIECE_AFTER_TILE = {}
for p, (j, off, w, need) in enumerate(PIECES):
    PIECE_AFTER_TILE.setdefault(need, []).append(p)

_cache = {}


def _build():
    nc = bacc.Bacc("TRN2", target_bir_lowering=False, debug=False)
    xi = nc.declare_dram_parameter("xi", [128, NTILE], mybir.dt.int32, isOutput=False)
    tbl = nc.declare_dram_parameter("tbl", [VOCAB, EMB], mybir.dt.float32, isOutput=False)
    wt = nc.declare_dram_parameter("wt", [128, WIN * OUT], MM_DT, isOutput=False)
    bv = nc.declare_dram_parameter("bias", [128, 1], mybir.dt.float32, isOutput=False)
    idm = nc.declare_dram_parameter("idm", [128, 128], MM_DT, isOutput=False)
    out = nc.declare_dram_parameter("out", [128, T], MM_DT, isOutput=True)

    idx_sb = nc.alloc_sbuf_tensor("idx_sb", [128, NTILE], mybir.dt.int32)
    gb = nc.alloc_sbuf_tensor("gb", [128, NTILE, EMB], MM_DT)
    embT = nc.alloc_sbuf_tensor("embT", [128, BPC * EC], MM_DT)
    wt_sb = nc.alloc_sbuf_tensor("wt_sb", [128, WIN * OUT], MM_DT)
    b_sb = nc.alloc_sbuf_tensor("b_sb", [128, 1], mybir.dt.float32)
    ident = nc.alloc_sbuf_tensor("ident", [128, 128], MM_DT)
    aos = [nc.alloc_sbuf_tensor(f"ao{i}", [128, SPAN], MM_DT) for i in range(NPS)]
    pts = [nc.alloc_psum_tensor(f"pt{i}", [128, 128], MM_DT) for i in range(NPT)]
    pss = [nc.alloc_psum_tensor(f"ps{i}", [128, SPAN], mybir.dt.float32) for i in range(NPS)]

    with (
        nc.semaphore("s_idx") as s_idx,
        nc.semaphore("s_ld") as s_ld,
        nc.semaphore("s_g") as s_g,
        nc.semaphore("s_t") as s_t,
        nc.semaphore("s_e") as s_e,
        nc.semaphore("s_mm") as s_mm,
        nc.semaphore("s_act") as s_act,
        nc.semaphore("s_out") as s_out,
        nc.Block(no_gpsimd_drain=True) as block,
    ):

        @block.sync
        def _(sync):
            sync.dma_start(out=idx_sb[:], in_=xi[:]).then_inc(s_idx, 16)
            sync.dma_start(out=wt_sb[:], in_=wt[:]).then_inc(s_ld, 16)
            sync.dma_start(out=b_sb[:], in_=bv[:]).then_inc(s_ld, 16)
            sync.dma_start(out=ident[:], in_=idm[:]).then_inc(s_ld, 16)
            for p, (j, off, w, _need) in enumerate(PIECES):
                sync.wait_ge(s_act, p + 1)
                sync.dma_start(
                    out=out[:, j * SPAN + off : j * SPAN + off + w],
                    in_=aos[p % NPS][:, 0:w],
                ).then_inc(s_out, 16)
            sync.wait_ge(s_out, 16 * len(PIECES))

        @block.vector
        def _(vector):
            for bb in range(BPC):
                nc.vector.memset(embT[:, bb * EC : bb * EC + HALO], 0.0)
                nc.vector.memset(embT[:, bb * EC + HALO + S : (bb + 1) * EC], 0.0)
            for c in range(NTILE):
                vector.wait_ge(s_t, c + 1)
                bb, tl = c // TPB, (c % TPB) * 128
                nc.vector.tensor_copy(
                    out=embT[:, bb * EC + HALO + tl : bb * EC + HALO + tl + 128],
                    in_=pts[c % NPT][:],
                ).then_inc(s_e, 1)

        @block.gpsimd
        def _(gpsimd):
            gpsimd.wait_ge(s_idx, 16)
            for c in range(NTILE):
                nc.gpsimd.indirect_dma_start(
                    out=gb[:, c, :],
                    out_offset=None,
                    in_=tbl[:],
                    in_offset=IndirectOffsetOnAxis(ap=idx_sb[:, c : c + 1], axis=0),
                ).then_inc(s_g, 16)

        @block.tensor
        def _(tensor):
            tensor.wait_ge(s_ld, 48)   # wt + bias + ident loaded
            for c in range(NTILE):
                tensor.wait_ge(s_g, 16 * (c + 1))
                if c >= NPT:
                    tensor.wait_ge(s_e, c - NPT + 1)   # pt bank free
                nc.tensor.transpose(
                    out=pts[c % NPT][:], in_=gb[:, c, :], identity=ident[:]
                ).then_inc(s_t, 1)
                for p in PIECE_AFTER_TILE.get(c, []):
                    j, off, w, need = PIECES[p]
                    tensor.wait_ge(s_e, need + 1)
                    if p >= NPS:
                        tensor.wait_ge(s_act, p - NPS + 1)   # ps bank free
                    bb, ts0 = j // SPB, (j % SPB) * SPAN
                    ps = pss[p % NPS]
                    for k in range(WIN):
                        mm = nc.tensor.matmul(
                            out=ps[:, 0:w],
                            lhsT=wt_sb[:, k * OUT : (k + 1) * OUT],
                            rhs=embT[:, bb * EC + ts0 + off + k : bb * EC + ts0 + off + k + w],
                            start=(k == 0),
                            stop=(k == WIN - 1),
                        )
                    mm.then_inc(s_mm, 1)

        @block.scalar
        def _(scalar):
            scalar.wait_ge(s_ld, 32)   # bias loaded
            for p, (j, off, w, _need) in enumerate(PIECES):
                scalar.wait_ge(s_mm, p + 1)
                if p >= NPS:
                    scalar.wait_ge(s_out, 16 * (p - NPS + 1))   # ao buffer free
                nc.scalar.activation(
                    out=aos[p % NPS][:, 0:w],
                    in_=pss[p % NPS][:, 0:w],
                    func=mybir.ActivationFunctionType.Gelu,
                    bias=b_sb[:, 0:1],
                ).then_inc(s_act, 1)

    nc.compile()
    return nc


def _prep_inputs(x, emb_table, W, b):
    import ml_dtypes

    x = np.asarray(x).astype(np.int32)
    emb_table = np.ascontiguousarray(np.asarray(emb_table, dtype=np.float32))
    W = np.asarray(W, dtype=np.float32)
    b = np.asarray(b, dtype=np.float32)
    wt = np.ascontiguousarray(
        W.reshape(OUT, WIN, EMB).transpose(2, 1, 0).reshape(EMB, WIN * OUT)
        .astype(ml_dtypes.bfloat16)
    )
    bias = np.ascontiguousarray(b.reshape(128, 1))
    idm = np.eye(128, dtype=ml_dtypes.bfloat16)
    in_maps = []
    for core in range(NCORES):
        flat = x[core * BPC : (core + 1) * BPC].reshape(-1)
        xic = np.ascontiguousarray(flat.reshape(NTILE, 128).T)
        in_maps.append({"xi": xic, "tbl": emb_table, "wt": wt, "bias": bias, "idm": idm})
    return in_maps


def kernel(x, emb_table, W, b, _trace=False):
    if "nc" not in _cache:
        _cache["nc"] = _build()
    nc = _cache["nc"]
    in_maps = _prep_inputs(x, emb_table, W, b)
    res = run_bass_kernel_spmd(nc, in_maps, core_ids=list(range(NCORES)), trace=_trace)
    _cache["last_result"] = res
    outs = []
    for core in range(NCORES):
        oc = np.asarray(res.results[core]["out"]).astype(np.float32)
        outs.append(oc.T.reshape(BPC, S, OUT))
    return np.concatenate(outs, axis=0)


# revision 9
# speedup vs baseline: 1.1496x; 1.0069x over previous
"""ConvEncoder kernel for 8 TRN2 NeuronCores (raw Bacc, manual semaphores).

Computes: emb = emb_table[x]; windows = im2col(pad(emb), WIN=5);
y = gelu(windows @ W.T + b), for x (16, 2048) int32 ids.
Sharding: data-parallel over batch - 2 batches per core x 8 cores; the
embedding table (f32) and bf16 weights are replicated per core.

The gather is Q7-descgen-bound (~10ns/row on every SWDGE path), so the
pipeline is built to keep the 32 per-tile indirect gathers streaming
back-to-back from the moment the ids land, with everything else hidden
under them:
  sync:   idx load first, then wt/bias/ident; per-piece output stores
  gpsimd: 32 indirect row-gathers (f32 rows -> bf16 in flight), offsets
          sliced directly from idx_sb (no per-column copies to wait on)
  tensor: 32 bf16 transposes + 5-tap matmuls per span (PSUM f32)
  vector: embT halo memsets, PSUM->embT copies
  scalar: exact GELU(+bias) PSUM -> bf16 ring (host upcasts to f32)
"""

import numpy as np

import concourse.bass as bass
import concourse.mybir as mybir
from concourse import bacc
from concourse.bass import IndirectOffsetOnAxis
from concourse.bass_utils import run_bass_kernel_spmd

B, S, EMB, WIN, OUT, VOCAB = 16, 2048, 128, 5, 128, 50257
NCORES = 8
BPC = B // NCORES
T = BPC * S                    # 4096 tokens/core
NTILE = T // 128               # 32
TPB = S // 128                 # 16
SPAN = 512
NSPAN = T // SPAN              # 8
SPB = S // SPAN                # 4
HALO = WIN // 2
EC = S + 2 * HALO              # 2052

MM_DT = mybir.dt.bfloat16
NPT = 5                        # transpose psum banks
NPS = 3                        # matmul psum banks / ao buffers

# output pieces: spans 0..6 full, span 7 split in halves
# (j, col offset within span, width, last embT tile needed)
PIECES = []
for j in range(NSPAN - 1):
    need = min(4 * j + 4, ((j // SPB) + 1) * TPB - 1)
    PIECES.append((j, 0, SPAN, need))
PIECES.append((NSPAN - 1, 0, SPAN // 2, 30))
PIECES.append((NSPAN - 1, SPAN // 2, SPAN // 2, 31))
PIECE_AFTER_TILE = {}
for p, (j, off, w, need) in enumerate(PIECES):
    PIECE_AFTER_TILE.setdefault(need, []).append(p)

_cache = {}


def _build():
    nc = bacc.Bacc("TRN2", target_bir_lowering=False, debug=False)
    xi = nc.declare_dram_parameter("xi", [128, NTILE], mybir.dt.int32, isOutput=False)
    tbl = nc.declare_dram_parameter("tbl", [VOCAB, EMB], mybir.dt.float32, isOutput=False)
    wt = nc.declare_dram_parameter("wt", [128, WIN * OUT], MM_DT, isOutput=False)
    bv = nc.declare_dram_parameter("bias", [128, 1], mybir.dt.float32, isOutput=False)
    idm = nc.declare_dram_parameter("idm", [128, 128], MM_DT, isOutput=False)
    out = nc.declare_dram_parameter("out", [128, T], MM_DT, isOutput=True)

    idx_sb = nc.alloc_sbuf_tensor("idx_sb", [128, NTILE], mybir.dt.int32)
    gb = nc.alloc_sbuf_tensor("gb", [128, NTILE, EMB], MM_DT)
    embT = nc.alloc_sbuf_tensor("embT", [128, BPC * EC], MM_DT)
    wt_sb = nc.alloc_sbuf_tensor("wt_sb", [128, WIN * OUT], MM_DT)
    b_sb = nc.alloc_sbuf_tensor("b_sb", [128, 1], mybir.dt.float32)
    ident = nc.alloc_sbuf_tensor("ident", [128, 128], MM_DT)
    aos = [nc.alloc_sbuf_tensor(f"ao{i}", [128, SPAN], MM_DT) for i in range(NPS)]
    pts = [nc.alloc_psum_tensor(f"pt{i}", [128, 128], MM_DT) for i in range(NPT)]
    pss = [nc.alloc_psum_tensor(f"ps{i}", [128, SPAN], mybir.dt.float32) for i in range(NPS)]

    with (
        nc.semaphore("s_idx") as s_idx,
        nc.semaphore("s_ld") as s_ld,
        nc.semaphore("s_g") as s_g,
        nc.semaphore("s_t") as s_t,
        nc.semaphore("s_e") as s_e,
        nc.semaphore("s_mm") as s_mm,
        nc.semaphore("s_act") as s_act,
        nc.semaphore("s_out") as s_out,
        nc.Block(no_gpsimd_drain=True) as block,
    ):

        @block.sync
        def _(sync):
            sync.dma_start(out=idx_sb[:], in_=xi[:]).then_inc(s_idx, 16)
            sync.dma_start(out=wt_sb[:], in_=wt[:]).then_inc(s_ld, 16)
            sync.dma_start(out=b_sb[:], in_=bv[:]).then_inc(s_ld, 16)
            sync.dma_start(out=ident[:], in_=idm[:]).then_inc(s_ld, 16)
            for p, (j, off, w, _need) in enumerate(PIECES):
                sync.wait_ge(s_act, p + 1)
                sync.dma_start(
                    out=out[:, j * SPAN + off : j * SPAN + off + w],
                    in_=aos[p % NPS][:, 0:w],
                ).then_inc(s_out, 16)
            sync.wait_ge(s_out, 16 * len(PIECES))

        @block.vector
        def _(vector):
            for bb in range(BPC):
                nc.vector.memset(embT[:, bb * EC : bb * EC + HALO], 0.0)
                nc.vector.memset(embT[:, bb * EC + HALO + S : (bb + 1) * EC], 0.0)
            for c in range(NTILE):
                vector.wait_ge(s_t, c + 1)
                bb, tl = c // TPB, (c % TPB) * 128
                nc.vector.tensor_copy(
                    out=embT[:, bb * EC + HALO + tl : bb * EC + HALO + tl + 128],
                    in_=pts[c % NPT][:],
                ).then_inc(s_e, 1)

        @block.gpsimd
        def _(gpsimd):
            gpsimd.wait_ge(s_idx, 16)
            for c in range(NTILE):
                nc.gpsimd.indirect_dma_start(
                    out=gb[:, c, :],
                    out_offset=None,
                    in_=tbl[:],
                    in_offset=IndirectOffsetOnAxis(ap=idx_sb[:, c : c + 1], axis=0),
                ).then_inc(s_g, 16)

        @block.tensor
        def _(tensor):
            tensor.wait_ge(s_ld, 48)   # wt + bias + ident loaded
            for c in range(NTILE):
                tensor.wait_ge(s_g, 16 * (c + 1))
                if c >= NPT:
                    tensor.wait_ge(s_e, c - NPT + 1)   # pt bank free
                nc.tensor.transpose(
                    out=pts[c % NPT][:], in_=gb[:, c, :], identity=ident[:]
                ).then_inc(s_t, 1)
                for p in PIECE_AFTER_TILE.get(c, []):
                    j, off, w, need = PIECES[p]
                    tensor.wait_ge(s_e, need + 1)
                    if p >= NPS:
                        tensor.wait_ge(s_act, p - NPS + 1)   # ps bank free
                    bb, ts0 = j // SPB, (j % SPB) * SPAN
                    ps = pss[p % NPS]
                    for k in range(WIN):
                        mm = nc.tensor.matmul(
                            out=ps[:, 0:w],
                            lhsT=wt_sb[:, k * OUT : (k + 1) * OUT],
                            rhs=embT[:, bb * EC + ts0 + off + k : bb * EC + ts0 + off + k + w],
                            start=(k == 0),
                            stop=(k == WIN - 1),
                        )
                    mm.then_inc(s_mm, 1)

        @block.scalar
        def _(scalar):
            scalar.wait_ge(s_ld, 32)   # bias loaded
            for p, (j, off, w, _need) in enumerate(PIECES):
                scalar.wait_ge(s_mm, p + 1)
                if p >= NPS:
                    scalar.wait_ge(s_out, 16 * (p - NPS + 1))   # ao buffer free
                nc.scalar.activation(
                    out=aos[p % NPS][:, 0:w],
                    in_=pss[p % NPS][:, 0:w],
                    func=mybir.ActivationFunctionType.Gelu,
                    bias=b_sb[:, 0:1],
                ).then_inc(s_act, 1)

    nc.compile()
    return nc


def _prep_inputs(x, emb_table, W, b):
    import ml_dtypes

    x = np.asarray(x).astype(np.int32)
    emb_table = np.ascontiguousarray(np.asarray(emb_table, dtype=np.float32))
    W = np.asarray(W, dtype=np.float32)
    b = np.asarray(b, dtype=np.float32)
    wt = np.ascontiguousarray(
        W.reshape(OUT, WIN, EMB).transpose(2, 1, 0).reshape(EMB, WIN * OUT)
        .astype(ml_dtypes.bfloat16)
    )
    bias = np.ascontiguousarray(b.reshape(128, 1))
    idm = np.eye(128, dtype=ml_dtypes.bfloat16)
    in_maps = []
    for core in range(NCORES):
        flat = x[core * BPC : (core + 1) * BPC].reshape(-1)
        xic = np.ascontiguousarray(flat.reshape(NTILE, 128).T)
        in_maps.append({"xi": xic, "tbl": emb_table, "wt": wt, "bias": bias, "idm": idm})
    return in_maps


def kernel(x, emb_table, W, b, _trace=False):
    if "nc" not in _cache:
        _cache["nc"] = _build()
    nc = _cache["nc"]
    in_maps = _prep_inputs(x, emb_table, W, b)
    res = run_bass_kernel_spmd(nc, in_maps, core_ids=list(range(NCORES)), trace=_trace)
    _cache["last_result"] = res
    outs = []
    for core in range(NCORES):
        oc = np.asarray(res.results[core]["out"]).astype(np.float32)
        outs.append(oc.T.reshape(BPC, S, OUT))
    return np.concatenate(outs, axis=0)
